# revision 3
# baseline (speedup 1.0000x reference)
"""MLA (multi-latent attention) Trainium2 kernel, 8-core SPMD.

Sharding: tensor-parallel over heads (4 groups of 4 heads) x data-parallel
over batch (2), = 8 cores. Low-rank a-projections are replicated; q_b/kv_b
output dims and out_proj input dim are sharded by head. Each core returns a
token-major partial out-projection [n, 2048]; the host sums the 4 head-group
partials per batch element (the standard row-parallel TP unshard).

On-chip layout is feature-major ("T" = [features on partitions, tokens on
free]) so every matmul contracts over the partition dim with natural layouts.
Attention keeps keys on partitions / queries on free: scoresT = kfT.T @ qfT,
pT = exp(scale * scoresT), out_avT = v.T-slices @ pT, with softmax
denominators accumulated by a ones-row matmul. No max subtraction (logits for
this distribution are O(10), far from fp32 exp overflow).
"""

from contextlib import ExitStack

import numpy as np
import ml_dtypes

import concourse.bacc as bacc
import concourse.mybir as mybir
from concourse.tile import TileContext
from concourse import bass_utils

BF16 = mybir.dt.bfloat16
F32 = mybir.dt.float32
NPBF16 = ml_dtypes.bfloat16

EMBED = 2048
HEADS = 16
NOPE = 128
VDIM = 128
ROPE = 64
Q_HEAD = NOPE + ROPE  # 192
KV_RANK = 512
BASE = 10000.0
SCALE = 1.0 / float(np.sqrt(Q_HEAD))

NH = 4          # heads per core
KC = EMBED // 128   # 16 k-chunks of the embedding dim
RC = KV_RANK // 128  # 4 k-chunks of the kv rank


def _emit(nc, n):
    """Trace the per-core kernel (same program on all 8 cores)."""
    TC = n // 512   # token chunks of 512
    NT = n // 128   # token chunks of 128
    AF = mybir.ActivationFunctionType
    OP = mybir.AluOpType

    # ---- DRAM I/O ----
    d_x = nc.dram_tensor("xT", [EMBED, n], BF16, kind="ExternalInput")
    d_wqa = nc.dram_tensor("wqa", [EMBED, 512], BF16, kind="ExternalInput")
    d_wkva = nc.dram_tensor("wkva", [EMBED, 576], BF16, kind="ExternalInput")
    d_wqb = nc.dram_tensor("wqb", [512, NH * Q_HEAD], BF16, kind="ExternalInput")
    d_wk = nc.dram_tensor("wk", [512, NH * NOPE], BF16, kind="ExternalInput")
    d_wv = nc.dram_tensor("wv", [512, NH * VDIM], BF16, kind="ExternalInput")
    d_wout = nc.dram_tensor("wout", [NH * VDIM, EMBED], BF16, kind="ExternalInput")
    d_cos = nc.dram_tensor("cosd", [128, n], F32, kind="ExternalInput")
    d_sin = nc.dram_tensor("sind", [128, n], F32, kind="ExternalInput")
    d_mask = nc.dram_tensor("maskd", [128, 4, 512], BF16, kind="ExternalInput")
    d_rotp = nc.dram_tensor("rotp", [128, 128], BF16, kind="ExternalInput")
    d_rotk = nc.dram_tensor("rotk", [64, 128], BF16, kind="ExternalInput")
    d_eyek = nc.dram_tensor("eyek", [64, 128], BF16, kind="ExternalInput")
    d_ones = nc.dram_tensor("onesd", [128, 1], BF16, kind="ExternalInput")
    d_out = nc.dram_tensor("out", [n, EMBED], F32, kind="ExternalOutput")

    with TileContext(nc) as tc, ExitStack() as st:
        # psum: "mm" slots shared by projections/rope/scores/out-proj,
        # plus dedicated av/den accumulators. 4 + 2 + 2 = 8 banks.
        psum = st.enter_context(tc.tile_pool(name="psum", bufs=1, space="PSUM"))

        def mm_ps(name):
            return psum.tile([128, 512], F32, tag="mm", bufs=4, name=name)

        # ---- mid pool: phase-1 outputs + rope constants (whole kernel) ----
        mid = st.enter_context(tc.tile_pool(name="mid", bufs=1))
        t_qa = mid.tile([128, RC, n], BF16)
        t_ckv = mid.tile([128, RC, n], BF16)
        t_kpr = mid.tile([64, n], BF16)  # raw k_pe (pre-rope)
        t_cos = mid.tile([128, n], F32)
        t_sin = mid.tile([128, n], F32)
        t_rotp = mid.tile([128, 128], BF16)
        t_rotk = mid.tile([64, 128], BF16)
        t_eyek = mid.tile([64, 128], BF16)
        nc.sync.dma_start(out=t_cos, in_=d_cos.ap())
        nc.sync.dma_start(out=t_sin, in_=d_sin.ap())
        nc.sync.dma_start(out=t_rotp, in_=d_rotp.ap())
        nc.sync.dma_start(out=t_rotk, in_=d_rotk.ap())
        nc.sync.dma_start(out=t_eyek, in_=d_eyek.ap())

        # ================= phase 1: qa = x@Wqa, ckv = x@Wkva =============
        with tc.tile_pool(name="ph1", bufs=1) as ph1:
            t_x = ph1.tile([128, KC, n], BF16)
            t_wqa = ph1.tile([128, KC, 512], BF16)
            t_wkva = ph1.tile([128, KC, 576], BF16)
            x_r = d_x.ap().rearrange("(c p) t -> p c t", p=128)
            nc.sync.dma_start(
                out=t_wqa, in_=d_wqa.ap().rearrange("(c p) m -> p c m", p=128)
            )
            nc.sync.dma_start(
                out=t_wkva, in_=d_wkva.ap().rearrange("(c p) m -> p c m", p=128)
            )

            for t in range(TC):
                ts = slice(t * 512, (t + 1) * 512)
                nc.sync.dma_start(out=t_x[:, :, ts], in_=x_r[:, :, ts])
                for m in range(4):  # qa chunks
                    ps = mm_ps("ps1")
                    for k in range(KC):
                        nc.tensor.matmul(
                            ps,
                            t_wqa[:, k, m * 128 : (m + 1) * 128],
                            t_x[:, k, ts],
                            start=(k == 0),
                            stop=(k == KC - 1),
                        )
                    nc.scalar.copy(t_qa[:, m, ts], ps)
                for m in range(4):  # compressed kv chunks
                    ps = mm_ps("ps2")
                    for k in range(KC):
                        nc.tensor.matmul(
                            ps,
                            t_wkva[:, k, m * 128 : (m + 1) * 128],
                            t_x[:, k, ts],
                            start=(k == 0),
                            stop=(k == KC - 1),
                        )
                    nc.scalar.copy(t_ckv[:, m, ts], ps)
                # k_pe chunk (64 wide)
                ps = mm_ps("ps3")
                for k in range(KC):
                    nc.tensor.matmul(
                        ps[:64],
                        t_wkva[:, k, 512:576],
                        t_x[:, k, ts],
                        start=(k == 0),
                        stop=(k == KC - 1),
                    )
                nc.scalar.copy(t_kpr[:, ts], ps[:64])

        # ---- attention-phase persistent tiles (after ph1 frees) ----
        attn_p = st.enter_context(tc.tile_pool(name="attn_p", bufs=1))
        t_qn = attn_p.tile([128, NH, n], BF16)
        t_qpe = attn_p.tile([128, 2, n], BF16)
        t_kn = attn_p.tile([128, NH, n], BF16)
        t_kpe = attn_p.tile([128, n], BF16)
        t_v = attn_p.tile([128, NT, NH * VDIM], BF16)
        t_ao = attn_p.tile([128, NH, n], BF16)
        t_wout = attn_p.tile([128, NH, EMBED], BF16)
        t_mask = attn_p.tile([128, 4, 512], BF16)
        t_ones = attn_p.tile([128, 1], BF16)
        nc.sync.dma_start(
            out=t_wout, in_=d_wout.ap().rearrange("(c p) f -> p c f", p=128)
        )
        nc.sync.dma_start(out=t_mask, in_=d_mask.ap())
        nc.sync.dma_start(out=t_ones, in_=d_ones.ap())

        # ============= phase 2: q/k/v projections ========================
        with tc.tile_pool(name="ph2", bufs=1) as ph2:
            t_wqb = ph2.tile([128, RC, NH * Q_HEAD], BF16)
            t_wk = ph2.tile([128, RC, NH * NOPE], BF16)
            t_wv = ph2.tile([128, RC, NH * VDIM], BF16)
            nc.sync.dma_start(
                out=t_wqb, in_=d_wqb.ap().rearrange("(c p) m -> p c m", p=128)
            )
            nc.sync.dma_start(
                out=t_wk, in_=d_wk.ap().rearrange("(c p) m -> p c m", p=128)
            )
            nc.sync.dma_start(
                out=t_wv, in_=d_wv.ap().rearrange("(c p) m -> p c m", p=128)
            )

            for t in range(TC):
                ts = slice(t * 512, (t + 1) * 512)
                for m in range(6):  # q: 4 nope chunks + 2 pe chunks
                    ps = mm_ps("psq")
                    for k in range(RC):
                        nc.tensor.matmul(
                            ps,
                            t_wqb[:, k, m * 128 : (m + 1) * 128],
                            t_qa[:, k, ts],
                            start=(k == 0),
                            stop=(k == RC - 1),
                        )
                    if m < 4:
                        nc.scalar.copy(t_qn[:, m, ts], ps)
                    else:
                        nc.scalar.copy(t_qpe[:, m - 4, ts], ps)
                for m in range(4):  # k_nope
                    ps = mm_ps("psk")
                    for k in range(RC):
                        nc.tensor.matmul(
                            ps,
                            t_wk[:, k, m * 128 : (m + 1) * 128],
                            t_ckv[:, k, ts],
                            start=(k == 0),
                            stop=(k == RC - 1),
                        )
                    nc.scalar.copy(t_kn[:, m, ts], ps)
            for mt in range(NT):  # v, token-major
                ps = mm_ps("psv")
                for k in range(RC):
                    nc.tensor.matmul(
                        ps,
                        t_ckv[:, k, mt * 128 : (mt + 1) * 128],
                        t_wv[:, k, :],
                        start=(k == 0),
                        stop=(k == RC - 1),
                    )
                nc.scalar.copy(t_v[:, mt, :], ps)

        # ================== RoPE =========================================
        with tc.tile_pool(name="ropep", bufs=4) as rp:
            for g in range(2):  # q_pe, two heads per 128-partition group
                for t in range(TC):
                    ts = slice(t * 512, (t + 1) * 512)
                    pr = mm_ps("prq")
                    nc.tensor.matmul(pr, t_rotp, t_qpe[:, g, ts])
                    tt1 = rp.tile([128, 512], F32, tag="tt1")
                    tt2 = rp.tile([128, 512], F32, tag="tt2")
                    nc.vector.tensor_tensor(tt1, pr, t_sin[:, ts], op=OP.mult)
                    nc.vector.tensor_tensor(
                        tt2, t_qpe[:, g, ts], t_cos[:, ts], op=OP.mult
                    )
                    nc.vector.tensor_tensor(t_qpe[:, g, ts], tt1, tt2, op=OP.add)
            for t in range(TC):  # k_pe: rope + duplicate into both halves
                ts = slice(t * 512, (t + 1) * 512)
                pr = mm_ps("prk")
                pd = mm_ps("pdk")
                nc.tensor.matmul(pr, t_rotk, t_kpr[:, ts])
                nc.tensor.matmul(pd, t_eyek, t_kpr[:, ts])
                tt1 = rp.tile([128, 512], F32, tag="tt1")
                tt2 = rp.tile([128, 512], F32, tag="tt2")
                nc.vector.tensor_tensor(tt1, pr, t_sin[:, ts], op=OP.mult)
                nc.vector.tensor_tensor(tt2, pd, t_cos[:, ts], op=OP.mult)
                nc.vector.tensor_tensor(t_kpe[:, ts], tt1, tt2, op=OP.add)

        # =================== attention + out-proj ========================
        with (
            tc.tile_pool(name="ptp", bufs=3) as ptp,
            tc.tile_pool(name="smallp", bufs=2) as smallp,
            tc.tile_pool(name="otp", bufs=3) as otp,
        ):
            for qb in range(TC):
                qs = slice(qb * 512, (qb + 1) * 512)
                nkb = 4 * qb + 4
                for h in range(NH):
                    ho = (h % 2) * 64
                    g = h // 2
                    ps_av = psum.tile([128, 512], F32, tag="av", bufs=2, name="psav")
                    ps_den = psum.tile([1, 512], F32, tag="den", bufs=2, name="psden")
                    for kb in range(nkb):
                        ks = slice(kb * 128, (kb + 1) * 128)
                        ps_s = mm_ps("pss")
                        nc.tensor.matmul(
                            ps_s, t_kn[:, h, ks], t_qn[:, h, qs],
                            start=True, stop=False,
                        )
                        nc.tensor.matmul(
                            ps_s,
                            t_kpe[ho : ho + 64, ks],
                            t_qpe[ho : ho + 64, g, qs],
                            start=False, stop=True,
                        )
                        pt = ptp.tile([128, 512], BF16, tag="pt")
                        nc.scalar.activation(pt, ps_s, AF.Exp, scale=SCALE)
                        if kb >= 4 * qb:
                            nc.vector.tensor_tensor(
                                pt, pt, t_mask[:, kb - 4 * qb, :], op=OP.mult
                            )
                        nc.tensor.matmul(
                            ps_av,
                            t_v[:, kb, h * VDIM : (h + 1) * VDIM],
                            pt,
                            start=(kb == 0),
                            stop=(kb == nkb - 1),
                        )
                        nc.tensor.matmul(
                            ps_den, t_ones, pt,
                            start=(kb == 0), stop=(kb == nkb - 1),
                        )
                    rec = smallp.tile([1, 512], F32, tag="rec")
                    nc.vector.reciprocal_approx_fast(rec, ps_den)
                    bc = smallp.tile([128, 512], F32, tag="bc")
                    nc.gpsimd.partition_broadcast(bc, rec)
                    nc.vector.tensor_tensor(t_ao[:, h, qs], ps_av, bc, op=OP.mult)
                # out-projection for this token block
                for mt in range(4):
                    tok = qb * 512 + mt * 128
                    for f in range(4):
                        ps_o = mm_ps("pso")
                        for h4 in range(NH):
                            nc.tensor.matmul(
                                ps_o,
                                t_ao[:, h4, tok : tok + 128],
                                t_wout[:, h4, f * 512 : (f + 1) * 512],
                                start=(h4 == 0),
                                stop=(h4 == NH - 1),
                            )
                        ot = otp.tile([128, 512], F32, tag="ot")
                        nc.vector.tensor_copy(ot, ps_o)
                        nc.sync.dma_start(
                            out=d_out.ap()[tok : tok + 128, f * 512 : (f + 1) * 512],
                            in_=ot,
                        )
    return nc


_NC_CACHE = {}


def build_mla(n=2048):
    if n not in _NC_CACHE:
        nc = bacc.Bacc(
            "TRN2",
            target_bir_lowering=False,
            debug=False,
            enable_asserts=False,
        )
        _emit(nc, n)
        nc.compile()
        _NC_CACHE[n] = nc
    return _NC_CACHE[n]


def make_host_inputs(x, Wqa, Wqb, Wkva, Wkvb, Wout, n):
    """Build the 8 per-core input maps (host-side sharding)."""
    # rope tables
    theta = BASE ** (-2.0 * np.arange(ROPE // 2, dtype=np.float32) / ROPE)
    pos = np.arange(n, dtype=np.float32)
    ang = pos[:, None] * theta[None, :]  # [n, 32]
    cos64 = np.repeat(np.cos(ang).T, 2, axis=0).astype(np.float32)  # [64, n]
    sin64 = np.repeat(np.sin(ang).T, 2, axis=0).astype(np.float32)
    cosd = np.tile(cos64, (2, 1))  # [128, n]
    sind = np.tile(sin64, (2, 1))

    kp = np.arange(128)[:, None, None]
    r = np.arange(4)[None, :, None]
    qf = np.arange(512)[None, None, :]
    maskd = (qf >= r * 128 + kp).astype(NPBF16)

    rot64 = np.zeros((64, 64), np.float32)
    for i in range(32):
        rot64[2 * i + 1, 2 * i] = -1.0
        rot64[2 * i, 2 * i + 1] = 1.0
    rotp = np.zeros((128, 128), np.float32)
    rotp[:64, :64] = rot64
    rotp[64:, 64:] = rot64
    rotk = np.hstack([rot64, rot64])
    eyek = np.hstack([np.eye(64, dtype=np.float32), np.eye(64, dtype=np.float32)])

    shared = {
        "wqa": Wqa.astype(NPBF16),
        "wkva": Wkva.astype(NPBF16),
        "cosd": cosd,
        "sind": sind,
        "maskd": maskd,
        "rotp": rotp.astype(NPBF16),
        "rotk": rotk.astype(NPBF16),
        "eyek": eyek.astype(NPBF16),
        "onesd": np.ones((128, 1), NPBF16),
    }
    Wqb_r = Wqb.reshape(512, HEADS, Q_HEAD)
    Wkvb_r = Wkvb.reshape(KV_RANK, HEADS, NOPE + VDIM)
    Wout_r = Wout.reshape(HEADS, VDIM, EMBED)

    in_maps = []
    xT = [np.ascontiguousarray(x[be].T).astype(NPBF16) for be in range(x.shape[0])]
    for c in range(8):
        be, hg = c // 4, c % 4
        hsel = slice(4 * hg, 4 * hg + NH)
        wqb = np.concatenate(
            [
                Wqb_r[:, hsel, :NOPE].reshape(512, NH * NOPE),
                Wqb_r[:, hsel, NOPE:].reshape(512, NH * ROPE),
            ],
            axis=1,
        ).astype(NPBF16)
        in_maps.append(
            {
                **shared,
                "xT": xT[be],
                "wqb": wqb,
                "wk": np.ascontiguousarray(
                    Wkvb_r[:, hsel, :NOPE].reshape(512, NH * NOPE)
                ).astype(NPBF16),
                "wv": np.ascontiguousarray(
                    Wkvb_r[:, hsel, NOPE:].reshape(512, NH * VDIM)
                ).astype(NPBF16),
                "wout": np.ascontiguousarray(
                    Wout_r[hsel].reshape(NH * VDIM, EMBED)
                ).astype(NPBF16),
            }
        )
    return in_maps


def kernel(x, Wqa, Wqb, Wkva, Wkvb, Wout, _trace=False):
    x = np.asarray(x)
    b, n, _ = x.shape
    nc = build_mla(n)
    in_maps = make_host_inputs(
        np.asarray(x),
        np.asarray(Wqa),
        np.asarray(Wqb),
        np.asarray(Wkva),
        np.asarray(Wkvb),
        np.asarray(Wout),
        n,
    )
    res = bass_utils.run_bass_kernel_spmd(
        nc, in_maps, core_ids=list(range(8)), trace=_trace
    )
    out = np.zeros((b, n, EMBED), np.float32)
    for c in range(8):
        out[c // 4] += res.results[c]["out"]
    if _trace:
        kernel.last_results = res
    return out


# revision 5
# speedup vs baseline: 1.0788x; 1.0788x over previous
"""MLA (multi-latent attention) Trainium2 kernel, 8-core SPMD.

Sharding: tensor-parallel over heads (4 groups of 4 heads) x data-parallel
over batch (2), = 8 cores. Low-rank a-projections are replicated; q_b/kv_b
output dims and out_proj input dim are sharded by head. Each core returns a
token-major partial out-projection [n, 2048]; the host sums the 4 head-group
partials per batch element (the standard row-parallel TP unshard).

On-chip layout is feature-major ("T" = [features on partitions, tokens on
free]) so every matmul contracts over the partition dim with natural layouts.
Attention keeps keys on partitions / queries on free: scoresT = kfT.T @ qfT
(+ a -30000*U causal-bias matmul on diagonal blocks), pT = exp(scale *
scoresT), out_avT = v-slices @ pT, with softmax denominators accumulated by a
ones-row matmul. No max subtraction (logits for this distribution are O(10),
far from fp32 exp overflow).
"""

from contextlib import ExitStack

import numpy as np
import ml_dtypes

import concourse.bacc as bacc
import concourse.mybir as mybir
from concourse.tile import TileContext
from concourse import bass_utils

BF16 = mybir.dt.bfloat16
F32 = mybir.dt.float32
NPBF16 = ml_dtypes.bfloat16

EMBED = 2048
HEADS = 16
NOPE = 128
VDIM = 128
ROPE = 64
Q_HEAD = NOPE + ROPE  # 192
KV_RANK = 512
BASE = 10000.0
SCALE = 1.0 / float(np.sqrt(Q_HEAD))
MASK_BIAS = -30000.0

NH = 4          # heads per core
KC = EMBED // 128   # 16 k-chunks of the embedding dim
RC = KV_RANK // 128  # 4 k-chunks of the kv rank

# psum bank budget: pss 3 + av 2 + pso 2 + den 1 = 8
_PS_TAGS = {"pss": 3, "av": 2, "pso": 2, "den": 1}
_PS_ROT = ["pss", "pss", "pss", "av", "av", "pso", "pso", "den"]


def _emit(nc, n):
    """Trace the per-core kernel (same program on all 8 cores)."""
    TC = n // 512   # token chunks of 512
    NT = n // 128   # token chunks of 128
    AF = mybir.ActivationFunctionType
    OP = mybir.AluOpType

    # ---- DRAM I/O ----
    d_x = nc.dram_tensor("xT", [EMBED, n], BF16, kind="ExternalInput")
    d_wqa = nc.dram_tensor("wqa", [EMBED, 512], BF16, kind="ExternalInput")
    d_wkva = nc.dram_tensor("wkva", [EMBED, 576], BF16, kind="ExternalInput")
    d_wqb = nc.dram_tensor("wqb", [512, NH * Q_HEAD], BF16, kind="ExternalInput")
    d_wk = nc.dram_tensor("wk", [512, NH * NOPE], BF16, kind="ExternalInput")
    d_wv = nc.dram_tensor("wv", [512, NH * VDIM], BF16, kind="ExternalInput")
    d_wout = nc.dram_tensor("wout", [NH * VDIM, EMBED], BF16, kind="ExternalInput")
    d_cos = nc.dram_tensor("cosd", [128, n], F32, kind="ExternalInput")
    d_sin = nc.dram_tensor("sind", [128, n], F32, kind="ExternalInput")
    d_mask = nc.dram_tensor("maskd", [128, 4, 512], BF16, kind="ExternalInput")
    d_negeye = nc.dram_tensor("negeye", [128, 128], BF16, kind="ExternalInput")
    d_rotp = nc.dram_tensor("rotp", [128, 128], BF16, kind="ExternalInput")
    d_rotk = nc.dram_tensor("rotk", [64, 128], BF16, kind="ExternalInput")
    d_eyek = nc.dram_tensor("eyek", [64, 128], BF16, kind="ExternalInput")
    d_ones = nc.dram_tensor("onesd", [128, 1], BF16, kind="ExternalInput")
    d_out = nc.dram_tensor("out", [n, EMBED], F32, kind="ExternalOutput")

    with TileContext(nc) as tc, ExitStack() as st:
        psum = st.enter_context(tc.tile_pool(name="psum", bufs=1, space="PSUM"))
        rot_i = [0]

        def ps_any(name):
            tag = _PS_ROT[rot_i[0] % 8]
            rot_i[0] += 1
            return psum.tile([128, 512], F32, tag=tag, bufs=_PS_TAGS[tag], name=name)

        def ps_tag(tag, name):
            return psum.tile([128, 512], F32, tag=tag, bufs=_PS_TAGS[tag], name=name)

        # ---- mid pool: phase-1 outputs + rope constants ----
        mid = st.enter_context(tc.tile_pool(name="mid", bufs=1))
        t_qa = mid.tile([128, RC, n], BF16)
        t_ckv = mid.tile([128, RC, n], BF16)
        t_kpr = mid.tile([64, n], BF16)  # raw k_pe (pre-rope)
        t_cos = mid.tile([128, n], F32)
        t_sin = mid.tile([128, n], F32)
        t_rotp = mid.tile([128, 128], BF16)
        t_rotk = mid.tile([64, 128], BF16)
        t_eyek = mid.tile([64, 128], BF16)

        # ---- phase-2 weights: prefetched during phase 1 ----
        ph2w = st.enter_context(tc.tile_pool(name="ph2w", bufs=1))
        t_wqb = ph2w.tile([128, RC, NH * Q_HEAD], BF16)
        t_wk = ph2w.tile([128, RC, NH * NOPE], BF16)
        t_wv = ph2w.tile([128, RC, NH * VDIM], BF16)

        # ================= phase 1: qa = x@Wqa, ckv = x@Wkva =============
        with tc.tile_pool(name="ph1", bufs=1) as ph1:
            t_x = ph1.tile([128, KC, n], BF16)
            t_wqa = ph1.tile([128, KC, 512], BF16)
            t_wkva = ph1.tile([128, KC, 576], BF16)
            x_r = d_x.ap().rearrange("(c p) t -> p c t", p=128)
            # critical-path loads first, spread over engine queues
            nc.sync.dma_start(out=t_x[:, :, 0:512], in_=x_r[:, :, 0:512])
            nc.scalar.dma_start(
                out=t_wqa, in_=d_wqa.ap().rearrange("(c p) m -> p c m", p=128)
            )
            nc.gpsimd.dma_start(
                out=t_wkva, in_=d_wkva.ap().rearrange("(c p) m -> p c m", p=128)
            )
            nc.gpsimd.dma_start(
                out=t_wqb, in_=d_wqb.ap().rearrange("(c p) m -> p c m", p=128)
            )
            nc.gpsimd.dma_start(
                out=t_wk, in_=d_wk.ap().rearrange("(c p) m -> p c m", p=128)
            )
            nc.gpsimd.dma_start(
                out=t_wv, in_=d_wv.ap().rearrange("(c p) m -> p c m", p=128)
            )
            nc.gpsimd.dma_start(out=t_cos, in_=d_cos.ap())
            nc.gpsimd.dma_start(out=t_sin, in_=d_sin.ap())
            nc.gpsimd.dma_start(out=t_rotp, in_=d_rotp.ap())
            nc.gpsimd.dma_start(out=t_rotk, in_=d_rotk.ap())
            nc.gpsimd.dma_start(out=t_eyek, in_=d_eyek.ap())

            for t in range(TC):
                ts = slice(t * 512, (t + 1) * 512)
                if t > 0:
                    nc.sync.dma_start(out=t_x[:, :, ts], in_=x_r[:, :, ts])
                for m in range(4):  # qa chunks
                    ps = ps_any("ps1")
                    for k in range(KC):
                        nc.tensor.matmul(
                            ps,
                            t_wqa[:, k, m * 128 : (m + 1) * 128],
                            t_x[:, k, ts],
                            start=(k == 0),
                            stop=(k == KC - 1),
                        )
                    nc.scalar.copy(t_qa[:, m, ts], ps)
                for m in range(4):  # compressed kv chunks
                    ps = ps_any("ps2")
                    for k in range(KC):
                        nc.tensor.matmul(
                            ps,
                            t_wkva[:, k, m * 128 : (m + 1) * 128],
                            t_x[:, k, ts],
                            start=(k == 0),
                            stop=(k == KC - 1),
                        )
                    nc.scalar.copy(t_ckv[:, m, ts], ps)
                # k_pe chunk (64 wide)
                ps = ps_any("ps3")
                for k in range(KC):
                    nc.tensor.matmul(
                        ps[:64],
                        t_wkva[:, k, 512:576],
                        t_x[:, k, ts],
                        start=(k == 0),
                        stop=(k == KC - 1),
                    )
                nc.scalar.copy(t_kpr[:, ts], ps[:64])

        # ---- attention-phase persistent tiles (after ph1 frees) ----
        attn_p = st.enter_context(tc.tile_pool(name="attn_p", bufs=1))
        t_qn = attn_p.tile([128, NH, n], BF16)
        t_qpe = attn_p.tile([128, 2, n], BF16)
        t_kn = attn_p.tile([128, NH, n], BF16)
        t_kpe = attn_p.tile([128, n], BF16)
        t_v = attn_p.tile([128, NT, NH * VDIM], BF16)
        t_ao = attn_p.tile([128, NH, n], BF16)
        t_wout = attn_p.tile([128, NH, EMBED], BF16)
        t_mask = attn_p.tile([128, 4, 512], BF16)
        t_negeye = attn_p.tile([128, 128], BF16)
        t_ones = attn_p.tile([128, 1], BF16)
        nc.scalar.dma_start(
            out=t_wout, in_=d_wout.ap().rearrange("(c p) f -> p c f", p=128)
        )
        nc.scalar.dma_start(out=t_mask, in_=d_mask.ap())
        nc.scalar.dma_start(out=t_negeye, in_=d_negeye.ap())
        nc.scalar.dma_start(out=t_ones, in_=d_ones.ap())

        # ==== phase 2: q/k/v projections + rope, interleaved per chunk ====
        with tc.tile_pool(name="ropep", bufs=4) as rp:

            def rope_q(g, t):
                ts = slice(t * 512, (t + 1) * 512)
                pr = ps_any("prq")
                nc.tensor.matmul(pr, t_rotp, t_qpe[:, g, ts])
                tt1 = rp.tile([128, 512], F32, tag="tt1")
                tt2 = rp.tile([128, 512], F32, tag="tt2")
                nc.vector.tensor_tensor(tt1, pr, t_sin[:, ts], op=OP.mult)
                nc.vector.tensor_tensor(
                    tt2, t_qpe[:, g, ts], t_cos[:, ts], op=OP.mult
                )
                nc.vector.tensor_tensor(t_qpe[:, g, ts], tt1, tt2, op=OP.add)

            def rope_k(t):
                ts = slice(t * 512, (t + 1) * 512)
                pr = ps_any("prk")
                pd = ps_any("pdk")
                nc.tensor.matmul(pr, t_rotk, t_kpr[:, ts])
                nc.tensor.matmul(pd, t_eyek, t_kpr[:, ts])
                tt1 = rp.tile([128, 512], F32, tag="tt1")
                tt2 = rp.tile([128, 512], F32, tag="tt2")
                nc.vector.tensor_tensor(tt1, pr, t_sin[:, ts], op=OP.mult)
                nc.vector.tensor_tensor(tt2, pd, t_cos[:, ts], op=OP.mult)
                nc.vector.tensor_tensor(t_kpe[:, ts], tt1, tt2, op=OP.add)

            for t in range(TC):
                ts = slice(t * 512, (t + 1) * 512)
                for m in range(6):  # q: 4 nope chunks + 2 pe chunks
                    ps = ps_any("psq")
                    for k in range(RC):
                        nc.tensor.matmul(
                            ps,
                            t_wqb[:, k, m * 128 : (m + 1) * 128],
                            t_qa[:, k, ts],
                            start=(k == 0),
                            stop=(k == RC - 1),
                        )
                    if m < 4:
                        nc.scalar.copy(t_qn[:, m, ts], ps)
                    else:
                        nc.scalar.copy(t_qpe[:, m - 4, ts], ps)
                rope_q(0, t)
                rope_q(1, t)
                for m in range(4):  # k_nope
                    ps = ps_any("psk")
                    for k in range(RC):
                        nc.tensor.matmul(
                            ps,
                            t_wk[:, k, m * 128 : (m + 1) * 128],
                            t_ckv[:, k, ts],
                            start=(k == 0),
                            stop=(k == RC - 1),
                        )
                    nc.scalar.copy(t_kn[:, m, ts], ps)
                rope_k(t)
                for mt in range(4 * t, 4 * t + 4):  # v, token-major
                    ps = ps_any("psv")
                    for k in range(RC):
                        nc.tensor.matmul(
                            ps,
                            t_ckv[:, k, mt * 128 : (mt + 1) * 128],
                            t_wv[:, k, :],
                            start=(k == 0),
                            stop=(k == RC - 1),
                        )
                    nc.scalar.copy(t_v[:, mt, :], ps)

        # =================== attention + out-proj ========================
        with (
            tc.tile_pool(name="ptp", bufs=4) as ptp,
            tc.tile_pool(name="smallp", bufs=2) as smallp,
            tc.tile_pool(name="otp", bufs=3) as otp,
        ):
            for qb in range(TC):
                qs = slice(qb * 512, (qb + 1) * 512)
                nkb = 4 * qb + 4
                for h in range(NH):
                    ho = (h % 2) * 64
                    g = h // 2
                    ps_av = ps_tag("av", "psav")
                    ps_den = ps_tag("den", "psden")
                    for kb in range(nkb):
                        ks = slice(kb * 128, (kb + 1) * 128)
                        diag = kb >= 4 * qb
                        ps_s = ps_tag("pss", "pss")
                        nc.tensor.matmul(
                            ps_s, t_kn[:, h, ks], t_qn[:, h, qs],
                            start=True, stop=False,
                        )
                        nc.tensor.matmul(
                            ps_s,
                            t_kpe[ho : ho + 64, ks],
                            t_qpe[ho : ho + 64, g, qs],
                            start=False, stop=not diag,
                        )
                        if diag:  # causal bias: -30000 * U_r
                            nc.tensor.matmul(
                                ps_s,
                                t_negeye,
                                t_mask[:, kb - 4 * qb, :],
                                start=False, stop=True,
                            )
                        pt = ptp.tile([128, 512], BF16, tag="pt")
                        nc.scalar.activation(pt, ps_s, AF.Exp, scale=SCALE)
                        nc.tensor.matmul(
                            ps_av,
                            t_v[:, kb, h * VDIM : (h + 1) * VDIM],
                            pt,
                            start=(kb == 0),
                            stop=(kb == nkb - 1),
                        )
                        nc.tensor.matmul(
                            ps_den[:1], t_ones, pt,
                            start=(kb == 0), stop=(kb == nkb - 1),
                        )
                    rec = smallp.tile([1, 512], F32, tag="rec")
                    nc.vector.reciprocal_approx_fast(rec, ps_den[:1])
                    bc = smallp.tile([128, 512], F32, tag="bc")
                    nc.gpsimd.partition_broadcast(bc, rec)
                    nc.vector.tensor_tensor(t_ao[:, h, qs], ps_av, bc, op=OP.mult)
                # out-projection for this token block
                for mt in range(4):
                    tok = qb * 512 + mt * 128
                    for f in range(4):
                        ps_o = ps_tag("pso", "pso")
                        for h4 in range(NH):
                            nc.tensor.matmul(
                                ps_o,
                                t_ao[:, h4, tok : tok + 128],
                                t_wout[:, h4, f * 512 : (f + 1) * 512],
                                start=(h4 == 0),
                                stop=(h4 == NH - 1),
                            )
                        ot = otp.tile([128, 512], F32, tag="ot")
                        nc.vector.tensor_copy(ot, ps_o)
                        nc.sync.dma_start(
                            out=d_out.ap()[tok : tok + 128, f * 512 : (f + 1) * 512],
                            in_=ot,
                        )
    return nc


_NC_CACHE = {}


def build_mla(n=2048):
    if n not in _NC_CACHE:
        nc = bacc.Bacc(
            "TRN2",
            target_bir_lowering=False,
            debug=False,
            enable_asserts=False,
        )
        _emit(nc, n)
        nc.compile()
        _NC_CACHE[n] = nc
    return _NC_CACHE[n]


def make_host_inputs(x, Wqa, Wqb, Wkva, Wkvb, Wout, n):
    """Build the 8 per-core input maps (host-side sharding)."""
    # rope tables
    theta = BASE ** (-2.0 * np.arange(ROPE // 2, dtype=np.float32) / ROPE)
    pos = np.arange(n, dtype=np.float32)
    ang = pos[:, None] * theta[None, :]  # [n, 32]
    cos64 = np.repeat(np.cos(ang).T, 2, axis=0).astype(np.float32)  # [64, n]
    sin64 = np.repeat(np.sin(ang).T, 2, axis=0).astype(np.float32)
    cosd = np.tile(cos64, (2, 1))  # [128, n]
    sind = np.tile(sin64, (2, 1))

    kp = np.arange(128)[:, None, None]
    r = np.arange(4)[None, :, None]
    qf = np.arange(512)[None, None, :]
    # U_r: 1.0 where EXCLUDED (future) -> biased by -30000 before exp
    maskd = (qf < r * 128 + kp).astype(NPBF16)
    negeye = (MASK_BIAS * np.eye(128, dtype=np.float32)).astype(NPBF16)

    rot64 = np.zeros((64, 64), np.float32)
    for i in range(32):
        rot64[2 * i + 1, 2 * i] = -1.0
        rot64[2 * i, 2 * i + 1] = 1.0
    rotp = np.zeros((128, 128), np.float32)
    rotp[:64, :64] = rot64
    rotp[64:, 64:] = rot64
    rotk = np.hstack([rot64, rot64])
    eyek = np.hstack([np.eye(64, dtype=np.float32), np.eye(64, dtype=np.float32)])

    shared = {
        "wqa": Wqa.astype(NPBF16),
        "wkva": Wkva.astype(NPBF16),
        "cosd": cosd,
        "sind": sind,
        "maskd": maskd,
        "negeye": negeye,
        "rotp": rotp.astype(NPBF16),
        "rotk": rotk.astype(NPBF16),
        "eyek": eyek.astype(NPBF16),
        "onesd": np.ones((128, 1), NPBF16),
    }
    Wqb_r = Wqb.reshape(512, HEADS, Q_HEAD)
    Wkvb_r = Wkvb.reshape(KV_RANK, HEADS, NOPE + VDIM)
    Wout_r = Wout.reshape(HEADS, VDIM, EMBED)

    in_maps = []
    xT = [np.ascontiguousarray(x[be].T).astype(NPBF16) for be in range(x.shape[0])]
    for c in range(8):
        be, hg = c // 4, c % 4
        hsel = slice(4 * hg, 4 * hg + NH)
        wqb = np.concatenate(
            [
                Wqb_r[:, hsel, :NOPE].reshape(512, NH * NOPE),
                Wqb_r[:, hsel, NOPE:].reshape(512, NH * ROPE),
            ],
            axis=1,
        ).astype(NPBF16)
        in_maps.append(
            {
                **shared,
                "xT": xT[be],
                "wqb": wqb,
                "wk": np.ascontiguousarray(
                    Wkvb_r[:, hsel, :NOPE].reshape(512, NH * NOPE)
                ).astype(NPBF16),
                "wv": np.ascontiguousarray(
                    Wkvb_r[:, hsel, NOPE:].reshape(512, NH * VDIM)
                ).astype(NPBF16),
                "wout": np.ascontiguousarray(
                    Wout_r[hsel].reshape(NH * VDIM, EMBED)
                ).astype(NPBF16),
            }
        )
    return in_maps


def kernel(x, Wqa, Wqb, Wkva, Wkvb, Wout, _trace=False):
    x = np.asarray(x)
    b, n, _ = x.shape
    nc = build_mla(n)
    in_maps = make_host_inputs(
        np.asarray(x),
        np.asarray(Wqa),
        np.asarray(Wqb),
        np.asarray(Wkva),
        np.asarray(Wkvb),
        np.asarray(Wout),
        n,
    )
    res = bass_utils.run_bass_kernel_spmd(
        nc, in_maps, core_ids=list(range(8)), trace=_trace
    )
    out = np.zeros((b, n, EMBED), np.float32)
    for c in range(8):
        out[c // 4] += res.results[c]["out"]
    if _trace:
        kernel.last_results = res
    return out


# revision 11
# speedup vs baseline: 1.2103x; 1.1219x over previous
"""MLA (multi-latent attention) Trainium2 kernel, 8-core SPMD.

Sharding: tensor-parallel over heads (4 groups of 4 heads) x data-parallel
over batch (2), = 8 cores. Low-rank a-projections are replicated; q_b/kv_b
output dims and out_proj input dim are sharded by head. Each core returns a
token-major partial out-projection [n, 2048]; the host sums the 4 head-group
partials per batch element (the standard row-parallel TP unshard).

On-chip layout is feature-major ("T" = [features on partitions, tokens on
free]) so every matmul contracts over the partition dim with natural layouts.
Attention keeps keys on partitions / queries on free: scoresT = kfT.T @ qfT
(+ a -30000*U causal-bias matmul on diagonal blocks), pT = exp(scale *
scoresT), out_avT = v-slices @ pT, with softmax denominators accumulated by a
ones-row matmul. No max subtraction (logits for this distribution are O(10),
far from fp32 exp overflow).
"""

from contextlib import ExitStack

import numpy as np
import ml_dtypes

import concourse.bacc as bacc
import concourse.mybir as mybir
from concourse.tile import TileContext
from concourse import bass_utils

BF16 = mybir.dt.bfloat16
F32 = mybir.dt.float32
NPBF16 = ml_dtypes.bfloat16

EMBED = 2048
HEADS = 16
NOPE = 128
VDIM = 128
ROPE = 64
Q_HEAD = NOPE + ROPE  # 192
KV_RANK = 512
BASE = 10000.0
SCALE = 1.0 / float(np.sqrt(Q_HEAD))
MASK_BIAS = -30000.0

NH = 4          # heads per core
KC = EMBED // 128   # 16 k-chunks of the embedding dim
RC = KV_RANK // 128  # 4 k-chunks of the kv rank

# psum bank budget: pss 3 + av 2 + pso 2 + den 1 = 8
_PS_TAGS = {"pss": 3, "av": 2, "pso": 2, "den": 1}
_PS_ROT = ["pss", "pss", "pss", "av", "av", "pso", "pso", "den"]


def _emit(nc, n):
    """Trace the per-core kernel (same program on all 8 cores)."""
    TC = n // 512   # token chunks of 512
    NT = n // 128   # token chunks of 128
    AF = mybir.ActivationFunctionType
    OP = mybir.AluOpType

    # ---- DRAM I/O ----
    d_x = nc.dram_tensor("xT", [128, TC, KC, 512], BF16, kind="ExternalInput")
    d_wqa = nc.dram_tensor("wqa", [128, KC, 512], BF16, kind="ExternalInput")
    d_wkva = nc.dram_tensor("wkva", [128, KC, 576], BF16, kind="ExternalInput")
    d_wqb = nc.dram_tensor("wqb", [128, RC, NH * Q_HEAD], BF16, kind="ExternalInput")
    d_wk = nc.dram_tensor("wk", [128, RC, NH * NOPE], BF16, kind="ExternalInput")
    d_wv = nc.dram_tensor("wv", [128, RC, NH * VDIM], BF16, kind="ExternalInput")
    d_wout = nc.dram_tensor("wout", [128, NH, EMBED], BF16, kind="ExternalInput")
    d_cos = nc.dram_tensor("cosd", [128, n], F32, kind="ExternalInput")
    d_sin = nc.dram_tensor("sind", [128, n], F32, kind="ExternalInput")
    d_mask = nc.dram_tensor("maskd", [128, 4, 512], BF16, kind="ExternalInput")
    d_negeye = nc.dram_tensor("negeye", [128, 128], BF16, kind="ExternalInput")
    d_rotp = nc.dram_tensor("rotp", [128, 128], BF16, kind="ExternalInput")
    d_rotk = nc.dram_tensor("rotk", [64, 128], BF16, kind="ExternalInput")
    d_eyek = nc.dram_tensor("eyek", [64, 128], BF16, kind="ExternalInput")
    d_ones = nc.dram_tensor("onesd", [128, 1], BF16, kind="ExternalInput")
    d_out = nc.dram_tensor("out", [n, EMBED], F32, kind="ExternalOutput")

    with TileContext(nc) as tc, ExitStack() as st:
        psum = st.enter_context(tc.tile_pool(name="psum", bufs=1, space="PSUM"))
        rot_i = [0]

        def ps_any(name):
            tag = _PS_ROT[rot_i[0] % 8]
            rot_i[0] += 1
            return psum.tile([128, 512], F32, tag=tag, bufs=_PS_TAGS[tag], name=name)

        def ps_tag(tag, name):
            return psum.tile([128, 512], F32, tag=tag, bufs=_PS_TAGS[tag], name=name)

        # ---- mid pool: phase-1 outputs + rope constants ----
        mid = st.enter_context(tc.tile_pool(name="mid", bufs=1))
        t_qa = mid.tile([128, RC, n], BF16)
        t_ckv = mid.tile([128, RC, n], BF16)
        t_kpr = mid.tile([64, n], BF16)  # raw k_pe (pre-rope)
        t_cos = mid.tile([128, n], F32)
        t_sin = mid.tile([128, n], F32)
        t_rotp = mid.tile([128, 128], BF16)
        t_rotk = mid.tile([64, 128], BF16)
        t_eyek = mid.tile([64, 128], BF16)

        # ---- phase-2 weights: prefetched during phase 1 ----
        ph2w = st.enter_context(tc.tile_pool(name="ph2w", bufs=1))
        t_wqb = ph2w.tile([128, RC, NH * Q_HEAD], BF16)
        t_wk = ph2w.tile([128, RC, NH * NOPE], BF16)
        t_wv = ph2w.tile([128, RC, NH * VDIM], BF16)

        # ================= phase 1: qa = x@Wqa, ckv = x@Wkva =============
        with tc.tile_pool(name="ph1", bufs=1) as ph1:
            t_x = ph1.tile([128, TC, KC, 512], BF16)
            t_wqa = ph1.tile([128, KC, 512], BF16)
            t_wkva = ph1.tile([128, KC, 576], BF16)
            # critical-path loads first, spread over engine queues
            nc.sync.dma_start(out=t_x[:, 0], in_=d_x.ap()[:, 0])
            nc.scalar.dma_start(out=t_wqa, in_=d_wqa.ap())
            nc.gpsimd.dma_start(out=t_wkva, in_=d_wkva.ap())
            nc.gpsimd.dma_start(out=t_wqb, in_=d_wqb.ap())
            nc.gpsimd.dma_start(out=t_wk, in_=d_wk.ap())
            nc.gpsimd.dma_start(out=t_wv, in_=d_wv.ap())
            nc.gpsimd.dma_start(out=t_cos, in_=d_cos.ap())
            nc.gpsimd.dma_start(out=t_sin, in_=d_sin.ap())
            nc.gpsimd.dma_start(out=t_rotp, in_=d_rotp.ap())
            nc.gpsimd.dma_start(out=t_rotk, in_=d_rotk.ap())
            nc.gpsimd.dma_start(out=t_eyek, in_=d_eyek.ap())

            for t in range(TC):
                ts = slice(t * 512, (t + 1) * 512)
                if t > 0:
                    nc.sync.dma_start(out=t_x[:, t], in_=d_x.ap()[:, t])
                for m in range(4):  # qa chunks
                    ps = ps_any("ps1")
                    for k in range(KC):
                        nc.tensor.matmul(
                            ps,
                            t_wqa[:, k, m * 128 : (m + 1) * 128],
                            t_x[:, t, k, :],
                            start=(k == 0),
                            stop=(k == KC - 1),
                        )
                    nc.scalar.copy(t_qa[:, m, ts], ps)
                for m in range(4):  # compressed kv chunks
                    ps = ps_any("ps2")
                    for k in range(KC):
                        nc.tensor.matmul(
                            ps,
                            t_wkva[:, k, m * 128 : (m + 1) * 128],
                            t_x[:, t, k, :],
                            start=(k == 0),
                            stop=(k == KC - 1),
                        )
                    nc.scalar.copy(t_ckv[:, m, ts], ps)
                # k_pe chunk (64 wide)
                ps = ps_any("ps3")
                for k in range(KC):
                    nc.tensor.matmul(
                        ps[:64],
                        t_wkva[:, k, 512:576],
                        t_x[:, t, k, :],
                        start=(k == 0),
                        stop=(k == KC - 1),
                    )
                nc.scalar.copy(t_kpr[:, ts], ps[:64])

        # ---- attention-phase persistent tiles (after ph1 frees) ----
        attn_p = st.enter_context(tc.tile_pool(name="attn_p", bufs=1))
        t_qn = attn_p.tile([128, NH, n], BF16)
        t_qpe = attn_p.tile([128, 2, n], BF16)
        t_kn = attn_p.tile([128, NH, n], BF16)
        t_kpe = attn_p.tile([128, n], BF16)
        t_v = attn_p.tile([128, NT, NH * VDIM], BF16)
        t_ao = attn_p.tile([128, NH, n], BF16)
        t_wout = attn_p.tile([128, NH, EMBED], BF16)
        t_mask = attn_p.tile([128, 4, 512], BF16)
        t_negeye = attn_p.tile([128, 128], BF16)
        nc.scalar.dma_start(out=t_wout, in_=d_wout.ap())
        nc.scalar.dma_start(out=t_mask, in_=d_mask.ap())
        nc.scalar.dma_start(out=t_negeye, in_=d_negeye.ap())
        t_ones = attn_p.tile([128, 1], BF16)
        nc.scalar.dma_start(out=t_ones, in_=d_ones.ap())

        # ==== phase 2: q/k/v projections + rope, interleaved per chunk ====
        with tc.tile_pool(name="ropep", bufs=4) as rp:

            def rope_q(g, t):
                ts = slice(t * 512, (t + 1) * 512)
                pr = ps_any("prq")
                nc.tensor.matmul(pr, t_rotp, t_qpe[:, g, ts])
                tt1 = rp.tile([128, 512], F32, tag="tt1")
                tt2 = rp.tile([128, 512], F32, tag="tt2")
                nc.vector.tensor_tensor(tt1, pr, t_sin[:, ts], op=OP.mult)
                nc.vector.tensor_tensor(
                    tt2, t_qpe[:, g, ts], t_cos[:, ts], op=OP.mult
                )
                nc.vector.tensor_tensor(t_qpe[:, g, ts], tt1, tt2, op=OP.add)

            def rope_k(t):
                ts = slice(t * 512, (t + 1) * 512)
                pr = ps_any("prk")
                pd = ps_any("pdk")
                nc.tensor.matmul(pr, t_rotk, t_kpr[:, ts])
                nc.tensor.matmul(pd, t_eyek, t_kpr[:, ts])
                tt1 = rp.tile([128, 512], F32, tag="tt1")
                tt2 = rp.tile([128, 512], F32, tag="tt2")
                nc.vector.tensor_tensor(tt1, pr, t_sin[:, ts], op=OP.mult)
                nc.vector.tensor_tensor(tt2, pd, t_cos[:, ts], op=OP.mult)
                nc.vector.tensor_tensor(t_kpe[:, ts], tt1, tt2, op=OP.add)

            for t in range(TC):
                ts = slice(t * 512, (t + 1) * 512)
                for m in range(6):  # q: 4 nope chunks + 2 pe chunks
                    ps = ps_any("psq")
                    for k in range(RC):
                        nc.tensor.matmul(
                            ps,
                            t_wqb[:, k, m * 128 : (m + 1) * 128],
                            t_qa[:, k, ts],
                            start=(k == 0),
                            stop=(k == RC - 1),
                        )
                    if m < 4:
                        nc.scalar.copy(t_qn[:, m, ts], ps)
                    else:
                        nc.scalar.copy(t_qpe[:, m - 4, ts], ps)
                rope_q(0, t)
                rope_q(1, t)
                for m in range(4):  # k_nope
                    ps = ps_any("psk")
                    for k in range(RC):
                        nc.tensor.matmul(
                            ps,
                            t_wk[:, k, m * 128 : (m + 1) * 128],
                            t_ckv[:, k, ts],
                            start=(k == 0),
                            stop=(k == RC - 1),
                        )
                    nc.scalar.copy(t_kn[:, m, ts], ps)
                rope_k(t)
                for mt in range(4 * t, 4 * t + 4):  # v, token-major
                    ps = ps_any("psv")
                    for k in range(RC):
                        nc.tensor.matmul(
                            ps,
                            t_ckv[:, k, mt * 128 : (mt + 1) * 128],
                            t_wv[:, k, :],
                            start=(k == 0),
                            stop=(k == RC - 1),
                        )
                    nc.scalar.copy(t_v[:, mt, :], ps)

        # =================== attention + out-proj ========================
        with (
            tc.tile_pool(name="ptp", bufs=4) as ptp,
            tc.tile_pool(name="smallp", bufs=2) as smallp,
            tc.tile_pool(name="otp", bufs=3) as otp,
        ):
            for qb in range(TC):
                qs = slice(qb * 512, (qb + 1) * 512)
                nkb = 4 * qb + 4
                for h in range(NH):
                    ho = (h % 2) * 64
                    g = h // 2
                    ps_av = ps_tag("av", "psav")
                    ps_den = ps_tag("den", "psden")
                    for kb in range(nkb):
                        ks = slice(kb * 128, (kb + 1) * 128)
                        diag = kb >= 4 * qb
                        r = kb - 4 * qb
                        ps_s = ps_tag("pss", "pss")
                        nc.tensor.matmul(
                            ps_s, t_kn[:, h, ks], t_qn[:, h, qs],
                            start=True, stop=False,
                        )
                        nc.tensor.matmul(
                            ps_s,
                            t_kpe[ho : ho + 64, ks],
                            t_qpe[ho : ho + 64, g, qs],
                            start=False, stop=not diag,
                        )
                        if diag:  # causal bias: -30000 * U_r
                            nc.tensor.matmul(
                                ps_s,
                                t_negeye,
                                t_mask[:, r, :],
                                start=False, stop=True,
                            )
                        pt = ptp.tile([128, 512], BF16, tag="pt")
                        nc.scalar.activation(pt, ps_s, AF.Exp, scale=SCALE)
                        nc.tensor.matmul(
                            ps_av,
                            t_v[:, kb, h * VDIM : (h + 1) * VDIM],
                            pt,
                            start=(kb == 0),
                            stop=(kb == nkb - 1),
                        )
                        nc.tensor.matmul(
                            ps_den[:1], t_ones, pt,
                            start=(kb == 0), stop=(kb == nkb - 1),
                        )
                    rec = smallp.tile([1, 512], F32, tag="rec")
                    nc.vector.reciprocal_approx_fast(rec, ps_den[:1])
                    bc = smallp.tile([128, 512], F32, tag="bc")
                    nc.gpsimd.partition_broadcast(bc, rec)
                    nc.vector.tensor_tensor(t_ao[:, h, qs], ps_av, bc, op=OP.mult)
                # out-projection for this token block
                for mt in range(4):
                    tok = qb * 512 + mt * 128
                    for f in range(4):
                        ps_o = ps_tag("pso", "pso")
                        for h4 in range(NH):
                            nc.tensor.matmul(
                                ps_o,
                                t_ao[:, h4, tok : tok + 128],
                                t_wout[:, h4, f * 512 : (f + 1) * 512],
                                start=(h4 == 0),
                                stop=(h4 == NH - 1),
                            )
                        ot = otp.tile([128, 512], F32, tag="ot")
                        nc.vector.tensor_copy(ot, ps_o)
                        nc.sync.dma_start(
                            out=d_out.ap()[tok : tok + 128, f * 512 : (f + 1) * 512],
                            in_=ot,
                        )
    return nc


_NC_CACHE = {}


def build_mla(n=2048):
    if n not in _NC_CACHE:
        nc = bacc.Bacc(
            "TRN2",
            target_bir_lowering=False,
            debug=False,
            enable_asserts=False,
        )
        _emit(nc, n)
        nc.compile()
        _NC_CACHE[n] = nc
    return _NC_CACHE[n]


def make_host_inputs(x, Wqa, Wqb, Wkva, Wkvb, Wout, n):
    """Build the 8 per-core input maps (host-side sharding)."""
    # rope tables
    theta = BASE ** (-2.0 * np.arange(ROPE // 2, dtype=np.float32) / ROPE)
    pos = np.arange(n, dtype=np.float32)
    ang = pos[:, None] * theta[None, :]  # [n, 32]
    cos64 = np.repeat(np.cos(ang).T, 2, axis=0).astype(np.float32)  # [64, n]
    sin64 = np.repeat(np.sin(ang).T, 2, axis=0).astype(np.float32)
    cosd = np.tile(cos64, (2, 1))  # [128, n]
    sind = np.tile(sin64, (2, 1))

    kp = np.arange(128)[:, None, None]
    r = np.arange(4)[None, :, None]
    qf = np.arange(512)[None, None, :]
    # U_r: 1.0 where EXCLUDED (future) -> biased by -30000 before exp
    maskd = (qf < r * 128 + kp).astype(NPBF16)
    negeye = (MASK_BIAS * np.eye(128, dtype=np.float32)).astype(NPBF16)

    rot64 = np.zeros((64, 64), np.float32)
    for i in range(32):
        rot64[2 * i + 1, 2 * i] = -1.0
        rot64[2 * i, 2 * i + 1] = 1.0
    rotp = np.zeros((128, 128), np.float32)
    rotp[:64, :64] = rot64
    rotp[64:, 64:] = rot64
    rotk = np.hstack([rot64, rot64])
    eyek = np.hstack([np.eye(64, dtype=np.float32), np.eye(64, dtype=np.float32)])

    def prelay(w, kc):
        # [kc*128, m] -> [128, kc, m] partition-major, contiguous
        return np.ascontiguousarray(
            w.reshape(kc, 128, w.shape[1]).transpose(1, 0, 2)
        ).astype(NPBF16)

    shared = {
        "wqa": prelay(Wqa, KC),
        "wkva": prelay(Wkva, KC),
        "cosd": cosd,
        "sind": sind,
        "maskd": maskd,
        "negeye": negeye,
        "onesd": np.ones((128, 1), NPBF16),
        "rotp": rotp.astype(NPBF16),
        "rotk": rotk.astype(NPBF16),
        "eyek": eyek.astype(NPBF16),
    }
    Wqb_r = Wqb.reshape(512, HEADS, Q_HEAD)
    Wkvb_r = Wkvb.reshape(KV_RANK, HEADS, NOPE + VDIM)
    Wout_r = Wout.reshape(HEADS, VDIM, EMBED)

    in_maps = []
    TC = n // 512
    # x[be].T -> [128, TC, KC, 512]: f=(c,p), t=(tb,tt)
    xT = [
        np.ascontiguousarray(
            x[be].T.reshape(KC, 128, TC, 512).transpose(1, 2, 0, 3)
        ).astype(NPBF16)
        for be in range(x.shape[0])
    ]
    for c in range(8):
        be, hg = c // 4, c % 4
        hsel = slice(4 * hg, 4 * hg + NH)
        wqb = prelay(
            np.concatenate(
                [
                    Wqb_r[:, hsel, :NOPE].reshape(512, NH * NOPE),
                    Wqb_r[:, hsel, NOPE:].reshape(512, NH * ROPE),
                ],
                axis=1,
            ),
            RC,
        )
        in_maps.append(
            {
                **shared,
                "xT": xT[be],
                "wqb": wqb,
                "wk": prelay(Wkvb_r[:, hsel, :NOPE].reshape(512, NH * NOPE), RC),
                "wv": prelay(Wkvb_r[:, hsel, NOPE:].reshape(512, NH * VDIM), RC),
                "wout": prelay(Wout_r[hsel].reshape(NH * VDIM, EMBED), NH),
            }
        )
    return in_maps


def kernel(x, Wqa, Wqb, Wkva, Wkvb, Wout, _trace=False):
    x = np.asarray(x)
    b, n, _ = x.shape
    nc = build_mla(n)
    in_maps = make_host_inputs(
        np.asarray(x),
        np.asarray(Wqa),
        np.asarray(Wqb),
        np.asarray(Wkva),
        np.asarray(Wkvb),
        np.asarray(Wout),
        n,
    )
    res = bass_utils.run_bass_kernel_spmd(
        nc, in_maps, core_ids=list(range(8)), trace=_trace
    )
    out = np.zeros((b, n, EMBED), np.float32)
    for c in range(8):
        out[c // 4] += res.results[c]["out"]
    if _trace:
        kernel.last_results = res
    return out
